# revision 2
# baseline (speedup 1.0000x reference)
"""Trainium2 Bass kernel: faithful-reshape causal attention, causal-skip version.

Per (b, h) block (x rows [128h, 128h+128) of batch b):
  qkv   = x_blk @ Wqkv                       # [128, 3072] (f32r gemm)
  pseudo-positions l = 16a + r; q/k/v[l] = qkv[a, 192r + {0,64,128} : +64]
Position-blocked causal attention in bf16:
  kpos block i = positions [128i, 128i+128) live on PARTITIONS via a
  double-transpose chain (qkv -> vT -> v_pos). S^T strip i = k_i^T q for
  qpos in [128i, 2048) only (causal skip: 53% of dense work). exp on ACT,
  single 128-wide diagonal affine_select per strip. PV accumulates per
  qpos block j over kpos blocks i<=j into [65, 512] PSUM (ones-augmented
  V gives denominators). Normalization: reciprocal + PE rank-1 broadcast.
  y_blk = (PV/denom) @ Wo (f32r).
32 independent blocks; 8 cores x 4 blocks, zero collectives.
"""
import sys

sys.path.insert(0, '/opt/trn_rl_repo')

import numpy as np

B, L, D = 2, 2048, 1024
H = 16              # heads == blocks per batch
RB = 128            # x rows per block
D3 = 3 * D
NR = 16             # r-groups (192-col chunks per row)
NB = 4              # blocks per core
NCORES = 8
P = 128
NKB = 16            # kpos blocks per (b,h) block

# pt strip i covers qpos [128i, 2048): width 2048-128i; offsets into ptb
STRIP_W = [2048 - 128 * i for i in range(NKB)]
STRIP_OFF = [0] * NKB
for _i in range(1, NKB):
    STRIP_OFF[_i] = STRIP_OFF[_i - 1] + STRIP_W[_i - 1]
PT_TOTAL = STRIP_OFF[-1] + STRIP_W[-1]   # 17408

_cached = {}


def _build_program():
    import concourse.bass as bass
    import concourse.mybir as mybir
    import concourse.tile as tile
    from concourse.tile import add_dep_helper

    f32 = mybir.dt.float32
    f32r = mybir.dt.float32r
    bf16 = mybir.dt.bfloat16
    EXP = mybir.ActivationFunctionType.Exp
    GE = mybir.AluOpType.is_ge

    nc = bass.Bass()
    xs = nc.declare_dram_parameter("xs", [NB, RB, D], f32, isOutput=False)
    wqkv = nc.declare_dram_parameter("wqkv", [D, D3], f32, isOutput=False)
    wo = nc.declare_dram_parameter("wo", [D, D], f32, isOutput=False)
    ys = nc.declare_dram_parameter("ys", [NB, RB, D], f32, isOutput=True)

    with tile.TileContext(nc) as tc:
        with (
            tc.tile_pool(name="const", bufs=1) as constp,
            tc.tile_pool(name="wq", bufs=2) as wqp,
            tc.tile_pool(name="wop", bufs=1) as wop,
            tc.tile_pool(name="xp", bufs=2) as xp,
            tc.tile_pool(name="yo", bufs=1) as yop,
            tc.tile_pool(name="xtp", bufs=1) as xtp,
            tc.tile_pool(name="qkvp", bufs=1) as qkvp,
            tc.tile_pool(name="qkt", bufs=2) as qktp,
            tc.tile_pool(name="vtt", bufs=1) as vtp,
            tc.tile_pool(name="vap", bufs=2) as vap,
            tc.tile_pool(name="ptp", bufs=2) as ptp,
            tc.tile_pool(name="wl", bufs=2) as wlp,
            tc.tile_pool(name="nrm", bufs=1) as nrmp,
            tc.tile_pool(name="ps", bufs=2, space="PSUM") as psp,
            tc.tile_pool(name="qps", bufs=2, space="PSUM") as qpsp,
            tc.tile_pool(name="ot", bufs=2, space="PSUM") as otp,
        ):
            def absorb_on(eng, *prods):
                # Walrus caps every instruction at ONE sync wait. Emit
                # queue-local nops that sync-depend on each producer; the
                # post-pass elides waits covered by these earlier nops
                # (queue dispatch is in-order, so an earlier wait gates all
                # later instructions in the same queue).
                for p in prods:
                    if p is None:
                        continue
                    n = eng.nop(hint="dep")
                    add_dep_helper(n.ins, p.ins, sync=True)

            def absorb(*prods):
                absorb_on(nc.tensor, *prods)

            ps_readers = []          # per ps-pool alloc: the op that evicts it
            ps_n = [0]
            dma_hs = []

            def ps_tile():
                n = ps_n[0]
                if n >= 2:
                    absorb(ps_readers[n - 2])
                ps_n[0] += 1
                return psp.tile([P, 1024], f32, tag="ps", name="pstile")

            qps_readers = []
            qps_n = [0]

            def qps_tile():
                n = qps_n[0]
                if n >= 2:
                    absorb(qps_readers[n - 2])
                qps_n[0] += 1
                return qpsp.tile([P, 512], f32, tag="qps", name="qpstile")

            ot_readers = []
            ot_n = [0]

            def ot_tile():
                n = ot_n[0]
                if n >= 2:
                    absorb(ot_readers[n - 2])
                ot_n[0] += 1
                return otp.tile([65, 512], f32, tag="ot", name="otq")

            identb_t = constp.tile([P, 128], bf16, tag="identb")
            identb = identb_t[:, 0:128]
            h_idb = nc.gpsimd.memset(identb, 0.0)
            absorb_on(nc.gpsimd, h_idb)
            h_idb2 = nc.gpsimd.affine_select(
                out=identb, in_=identb, compare_op=mybir.AluOpType.not_equal,
                fill=1.0, base=0, pattern=[[-1, 128]], channel_multiplier=1)
            ones_f = constp.tile([P, 128], f32, tag="onesf")
            nc.gpsimd.memset(ones_f[:], 1.0)
            onesr = constp.tile([1, 128], f32r, tag="onesr")
            h_ones = nc.vector.tensor_copy(onesr[:], ones_f[0:1, 0:128])
            absorb(h_ones)


            def emit_qkv_batch(i, g, qT, kT, vT_t):
                """Transpose r-groups [8g, 8g+8) of block i: 8 qk transposes
                into one PSUM tile + one DVE evict per q/k half, then 8 kv
                transposes + one Pool evict of the v half."""
                tp = qps_tile()[:, 0:512].bitcast(bf16)
                for t in range(8):
                    r = 8 * g + t
                    nc.tensor.transpose(
                        tp[:, 128 * t:128 * t + 128],
                        qkv[:, i, 192 * r:192 * r + 128], identb)
                src_rc = tp.rearrange("c (r a) -> c r a", r=8).rearrange(
                    "c r a -> c a r")
                nc.vector.tensor_copy(qT[:, :, 8 * g:8 * g + 8], src_rc[0:64])
                h_qk = nc.vector.tensor_copy(kT[:, :, 8 * g:8 * g + 8],
                                             src_rc[64:128])
                qps_readers.append(h_qk)
                tp2 = qps_tile()[:, 0:512].bitcast(bf16)
                h_tr2 = None
                for t in range(8):
                    r = 8 * g + t
                    h_tr2 = nc.tensor.transpose(
                        tp2[:, 128 * t:128 * t + 128],
                        qkv[:, i, 192 * r + 64:192 * r + 192], identb)
                absorb_on(nc.vector, h_tr2)
                h_vt = nc.vector.tensor_copy(
                    vT_t[:, :, 8 * g:8 * g + 8],
                    tp2.rearrange("c (r a) -> c r a", r=8).rearrange(
                        "c r a -> c a r")[64:128])
                qps_readers.append(h_vt)
                return h_qk, h_vt

            def emit_vaug(i, vT_t, h_vt_last, h_gate_pe=None):
                """Build v_aug[i]: position-major V + ones col via 16 batched
                PE transposes of vT slices; Pool evicts."""
                va = vap.tile([P, NR, 65], bf16, tag="va", name="va")
                absorb_on(nc.gpsimd, h_gate_pe)
                h_ms = nc.gpsimd.memset(va[:, :, 64:65], 1.0)
                absorb(h_vt_last, h_ms)
                h_vp = None
                for g in range(2):
                    tp3 = qps_tile()[:, 0:256].bitcast(bf16)
                    h_tr3 = None
                    for t in range(8):
                        i2 = 8 * g + t
                        h_tr3 = nc.tensor.transpose(
                            tp3[:, 64 * t:64 * t + 64],
                            vT_t[:, 8 * i2:8 * i2 + 8, :],
                            identb[0:64, 0:64])
                    absorb_on(nc.vector, h_tr3)
                    h_vp = nc.vector.tensor_copy(
                        va[:, 8 * g:8 * g + 8, 0:64],
                        tp3.rearrange("c (k f) -> c k f", k=8))
                    qps_readers.append(h_vp)
                return va, h_vp

            stg_hist = []
            # ---- phase A: x load + transpose; stream Wqkv, qkv gemm per block
            xT = xtp.tile([P, NB, 8, P], bf16, tag="xT")
            x_hist = []
            for i in range(NB):
                if len(x_hist) >= 2:
                    absorb_on(nc.gpsimd, *x_hist[-2])
                x_sb = xp.tile([P, D], f32, tag="x")
                h_x = nc.gpsimd.dma_start(x_sb[:], xs[i])
                dma_hs.append(h_x)
                if len(x_hist) >= 1:
                    absorb_on(nc.vector, x_hist[-1][1])
                x_bf = xp.tile([P, D], bf16, tag="xbf")
                h_xc = nc.vector.tensor_copy(x_bf[:], x_sb[:])
                absorb(h_xc)
                tp = qps_tile()[:, 0:512].bitcast(bf16)
                h_tr = None
                for k in range(8):
                    h_tr = nc.tensor.transpose(
                        tp[:, 128 * k:128 * k + 128],
                        x_bf[:, 128 * k:128 * k + 128], identb)
                h_e = nc.vector.tensor_copy(
                    xT[:, i, :, :], tp.rearrange("c (k a) -> c k a", k=8))
                qps_readers.append(h_e)
                x_hist.append((h_x, h_tr, h_e))

            qkv = qkvp.tile([P, NB, D3], bf16, tag="qkv")
            evict_h = {}
            wq_hist = []          # last PE reader of each stg chunk
            blk = {}              # per block: setup handles
            # block-0 setup interleaved with streaming: r-group r of block 0
            # becomes transposable once its qkv cols exist.
            rs_ready_qk = {}
            rs_ready_v = {}
            for r in range(NR):
                rs_ready_qk.setdefault((192 * r + 127) // 256, []).append(r)
                rs_ready_v.setdefault((192 * r + 191) // 256, []).append(r)

            qT0 = qktp.tile([64, P, NR], bf16, tag="qT", name="qT0")
            kT0 = qktp.tile([64, P, NR], bf16, tag="kT", name="kT0")
            vT0 = vtp.tile([64, P, NR], bf16, tag="vT", name="vT0")
            h_qkt0 = None
            h_vt0 = None
            del rs_ready_qk, rs_ready_v
            for nch in range(12):
                if len(stg_hist) >= 2:
                    absorb_on(nc.gpsimd, *stg_hist[-2])
                stg = wqp.tile([P, 8, 256], f32, tag="stg")
                h_sd = nc.gpsimd.dma_start(
                    stg[:],
                    wqkv.rearrange("(kc p) n -> p kc n", p=P)[:, :, 256 * nch:256 * nch + 256],
                )
                dma_hs.append(h_sd)
                cast_eng = nc.vector if nch % 2 else nc.gpsimd
                if len(wq_hist) >= 2:
                    absorb_on(cast_eng, wq_hist[-2])
                wq_t = wqp.tile([P, 8, 256], bf16, tag="wqt")
                h_cast = cast_eng.tensor_copy(wq_t[:], stg[:])
                absorb(h_cast)
                h_mm = None
                for i in range(NB):
                    qp = qps_tile()
                    for k in range(8):
                        h_mm = nc.tensor.matmul(
                            qp[:, 0:256],
                            xT[:, i, k, :],
                            wq_t[:, k, :],
                            start=(k == 0), stop=(k == 7),
                        )
                    h_ev = nc.vector.tensor_copy(
                        qkv[:, i, 256 * nch:256 * nch + 256], qp[:, 0:256])
                    qps_readers.append(h_ev)
                    evict_h[(nch, i)] = h_ev
                wq_hist.append(h_mm)
                stg_hist.append((h_sd, h_cast))
                # block-0 transposes once their r-groups' columns exist:
                # r 0-7 complete at chunk 5, r 8-15 at chunk 11
                if nch in (5, 11):
                    absorb(evict_h[(nch, 0)])
                    h_qkt0, h_vt0 = emit_qkv_batch(0, 0 if nch == 5 else 1,
                                                   qT0, kT0, vT0)

            # Wo streamed through the stg pool, cast to bf16
            wo_bf = wop.tile([P, 8, D], bf16, tag="wo")
            for wc in range(4):
                if len(stg_hist) >= 2:
                    absorb_on(nc.gpsimd, *stg_hist[-2])
                stg = wqp.tile([P, 8, 256], f32, tag="stg")
                h_wd = nc.gpsimd.dma_start(
                    stg[:],
                    wo.rearrange("(kc p) n -> p kc n", p=P)[:, :, 256 * wc:256 * wc + 256],
                )
                dma_hs.append(h_wd)
                cast_eng = nc.vector if wc % 2 else nc.gpsimd
                h_wc = cast_eng.tensor_copy(wo_bf[:, :, 256 * wc:256 * wc + 256], stg[:])
                stg_hist.append((h_wd, h_wc))

            # ---- phase B: per-block attention + output projection
            blk_pv_last = {}      # block -> last PV matmul handle
            blk_aff_last = {}     # block -> last affine_select handle
            y_hist = []
            mul_hist = []
            bc_hist = []
            pending_tail = []
            tail_last = {}
            tail_womm = {}        # block -> its tail's last Wo matmul

            def setup_block(i):
                _setup_block_body(i)

            def _setup_block_body(i):
                # WAR gate: qkT/vT bufs rotate every 2 blocks; their old
                # readers are block i-2's PE instructions.
                gate = blk_pv_last.get(i - 2)
                absorb_on(nc.vector, gate)
                absorb(evict_h[(11, i)])
                qT = qktp.tile([64, P, NR], bf16, tag="qT", name="qT")
                kT = qktp.tile([64, P, NR], bf16, tag="kT", name="kT")
                vT_t = vtp.tile([64, P, NR], bf16, tag="vT", name="vT")
                h_qk = h_vt = None
                for g in range(2):
                    h_qk, h_vt = emit_qkv_batch(i, g, qT, kT, vT_t)
                va, h_vp = emit_vaug(i, vT_t, h_vt, blk_pv_last.get(i - 2))
                blk[i] = ((qT, kT), va, h_qk, h_vp)

            def emit_tail(ti, t_wo_lhsT, t_h_mul):
                yp = ps_tile()
                absorb(t_h_mul)
                for n2 in range(2):
                    for k in range(8):
                        tail_last["womm"] = nc.tensor.matmul(
                            yp[:, 512 * n2:512 * n2 + 512],
                            t_wo_lhsT[:, k, :],
                            wo_bf[:, k, 512 * n2:512 * n2 + 512],
                            start=(k == 0), stop=(k == 7),
                        )
                tail_womm[ti] = tail_last["womm"]
                if len(y_hist) >= 1:
                    absorb_on(nc.gpsimd, *y_hist[-1])
                    absorb_on(nc.vector, *y_hist[-1])
                y_sb = yop.tile([P, D], f32, tag="y")
                h_ye = nc.vector.tensor_copy(y_sb[:], yp[:])
                tail_last["ye"] = h_ye
                ps_readers.append(h_ye)
                h_yd = nc.gpsimd.dma_start(ys[ti], y_sb[:])
                dma_hs.append(h_yd)
                y_hist.append((h_ye, h_yd))

            va0, h_vp0 = emit_vaug(0, vT0, h_vt0, None)
            blk[0] = ((qT0, kT0), va0, h_qkt0, h_vp0)

            h_exp = h_aff = h_mul = h_pv = None
            bst = {}   # per-block attention state

            def emit_strip(i, j):
                nonlocal h_exp, h_aff
                st_i = bst[i]
                qT, kT = blk[i][0]
                ptb = st_i["ptb"]
                w = STRIP_W[j]
                off = STRIP_OFF[j]
                lhsT = kT[:, 8 * j:8 * j + 8, :]
                h_exp_first = None
                h_exp_last = None
                h_aff_s = None
                col = 0
                ti = 0
                while col < w:
                    tw = min(1024, w - col)
                    st = ps_tile()
                    u = 0
                    while u < tw:
                        uw = min(512, tw - u)
                        a0 = 8 * j + (col + u) // 16
                        nc.tensor.matmul(
                            st[:, u:u + uw],
                            lhsT,
                            qT[:, a0:a0 + uw // 16, :],
                            start=True, stop=True,
                        )
                        u += uw
                    # WAR: ptb buffer reuse vs block i-2 PV/affine touches
                    if ti == 0 and j == 0:
                        absorb_on(nc.scalar, blk_pv_last.get(i - 2),
                                  blk_aff_last.get(i - 2))
                        absorb_on(nc.gpsimd, blk_pv_last.get(i - 2))
                    h_exp = nc.scalar.activation(
                        ptb[:, off + col:off + col + tw], st[:, 0:tw],
                        EXP, scale=0.25)
                    ps_readers.append(h_exp)
                    if ti == 0:
                        h_exp_first = h_exp
                        # diagonal causal mask on first 128 cols
                        absorb_on(nc.gpsimd, h_exp)
                        h_aff_s = nc.gpsimd.affine_select(
                            out=ptb[:, off:off + 128],
                            in_=ptb[:, off:off + 128],
                            compare_op=GE, fill=0.0, base=0,
                            pattern=[[1, 128]], channel_multiplier=-1)
                    h_exp_last = h_exp
                    col += tw
                    ti += 1
                h_aff = h_aff_s
                blk_aff_last[i] = h_aff_s
                st_i["strip_done"][j] = (h_exp_first, h_aff_s, h_exp_last)

            def emit_pv(i, j):
                nonlocal h_pv
                st_i = bst[i]
                v_aug = blk[i][1]
                ptb = st_i["ptb"]
                if j % 4 == 0:
                    st_i["ot_s"] = ot_tile()
                ot_s = st_i["ot_s"]
                he, ha, hl = st_i["strip_done"][j]
                absorb(ha, hl)
                for i2 in range(j + 1):
                    h_pv = nc.tensor.matmul(
                        ot_s[:, 128 * (j % 4):128 * (j % 4) + 128],
                        v_aug[:, i2, :],
                        ptb[:, STRIP_OFF[i2] + 128 * (j - i2):
                            STRIP_OFF[i2] + 128 * (j - i2) + 128],
                        start=(i2 == 0), stop=(i2 == j),
                    )
                blk_pv_last[i] = h_pv

            def emit_norm(i, s):
                nonlocal h_mul
                st_i = bst[i]
                ot_s = st_i["ot_s"]
                wo_lhsT = st_i["wo_lhsT"]
                rcp = nrmp.tile([1, 512], f32r, tag="rcp")
                absorb_on(nc.vector, bc_hist[-1] if bc_hist else None)
                with nc.allow_low_precision(reason="f32r rounding of 1/d"):
                    h_rcp = nc.vector.reciprocal(rcp[:], ot_s[64:65, :])
                bc = qps_tile()
                absorb(h_rcp)
                h_bc = nc.tensor.matmul(
                    bc[:, 0:512], onesr[:], rcp[:, 0:512],
                    start=True, stop=True,
                )
                bc_hist.append(h_bc)
                if mul_hist:
                    absorb_on(nc.vector, mul_hist[-1])
                bc_sb = nrmp.tile([64, 512], f32, tag="bc")
                h_bcc = nc.vector.tensor_copy(bc_sb[:], bc[0:64, 0:512])
                qps_readers.append(h_bcc)
                ot_rc = ot_s[0:64, :].rearrange("c (a r) -> c r a", r=NR)
                bc_rc = bc_sb[:].rearrange("c (a r) -> c r a", r=NR)
                nc.vector.tensor_mul(
                    wo_lhsT[0:64, :, 32 * s:32 * s + 32],
                    ot_rc[:, 0:16:2, :], bc_rc[:, 0:16:2, :])
                h_mul = nc.vector.tensor_mul(
                    wo_lhsT[64:128, :, 32 * s:32 * s + 32],
                    ot_rc[:, 1:16:2, :], bc_rc[:, 1:16:2, :])
                st_i["h_mul"] = h_mul
                mul_hist.append(h_mul)
                ot_readers.append(h_mul)

            seq = [(i, j) for i in range(NB) for j in range(NKB)]
            for idx, (i, j) in enumerate(seq):
                if j == 0:
                    # WAR: wlp bufs=2 — block i reuses block i-2's buffer,
                    # read by that block's Wo tail matmuls.
                    absorb_on(nc.vector, tail_womm.get(i - 2))
                    bst[i] = {
                        "strip_done": [None] * NKB,
                        "ot_s": None,
                        "h_mul": None,
                        "wo_lhsT": wlp.tile([P, 8, P], bf16, tag="wl",
                                            name="wo_lhsT"),
                        "ptb": ptp.tile([P, PT_TOTAL], bf16, tag="ptb",
                                        name="ptb"),
                    }
                    absorb(blk[i][2], blk[i][3])
                emit_strip(i, j)
                if idx >= 1:
                    pi, pj = seq[idx - 1]
                    emit_pv(pi, pj)
                    if pj % 4 == 3:
                        emit_norm(pi, pj // 4)
                    if pj == NKB - 1:
                        pending_tail.append(
                            (pi, bst[pi]["wo_lhsT"], bst[pi]["h_mul"]))
                if j == 6 and i + 1 < NB:
                    setup_block(i + 1)
                if j == 3 and pending_tail:
                    emit_tail(*pending_tail.pop(0))
            pi, pj = seq[-1]
            emit_pv(pi, pj)
            emit_norm(pi, 3)
            h_mul = bst[pi]["h_mul"]
            pending_tail.append((pi, bst[pi]["wo_lhsT"], h_mul))

            while pending_tail:
                emit_tail(*pending_tail.pop(0))

            # absorb the kernel-tail drain's dependencies onto SP nops
            absorb_on(nc.sync, *dma_hs)
            absorb_on(nc.sync, h_aff, h_exp, tail_last["ye"], tail_last["womm"],
                      h_mul, h_pv, h_idb2)

    return nc


def _elide_covered_waits(nc):
    """Walrus rejects >1 sync wait per instruction. Each queue's sequencer
    processes waits in dispatch order, so a wait already issued earlier in
    the same queue gates every later instruction in that queue. Drop waits
    that an earlier same-queue instruction (incl. absorber nops) covers."""
    observed = {}   # engine -> {sem_id: max waited value}
    leftover = []
    for inst in nc.all_instructions():
        si = inst.sync_info
        if si is None:
            continue
        if type(inst).__name__ in ("InstEventSemaphore", "InstTrigger"):
            continue  # barrier-protocol handshakes, not data waits
        eng = str(inst.engine)
        own = eng.split(".")[-1] + "_"
        obs = observed.setdefault(eng, {})
        ow = list(si.on_wait or [])
        keep = []
        for w in ow:
            if w.ant_name.startswith(own):
                # own-queue wait: satisfied by in-order dispatch
                continue
            if obs.get(w.id, -1) >= w.wait_value:
                continue
            keep.append(w)
            obs[w.id] = max(obs.get(w.id, -1), w.wait_value)
        if len(keep) != len(ow):
            si.on_wait = keep
            inst.sync_info = si
        if len(keep) > 1:
            leftover.append((inst.name, type(inst).__name__, eng,
                             [(w.ant_name, w.wait_value) for w in keep]))
    if leftover:
        import logging
        logging.warning("multi-wait instructions remain: %s", leftover[:12])


def _get_program():
    if "nc" not in _cached:
        nc = _build_program()
        _elide_covered_waits(nc)
        _cached["nc"] = nc
    return _cached["nc"]


def kernel(x=None, mask=None, Wqkv=None, Wo=None, **_ignored):
    """Full inputs -> full output. mask is ignored (guaranteed causal tril)."""
    from concourse.bass_utils import run_bass_kernel_spmd

    x = np.ascontiguousarray(np.asarray(x, dtype=np.float32))
    Wqkv = np.ascontiguousarray(np.asarray(Wqkv, dtype=np.float32))
    Wo = np.ascontiguousarray(np.asarray(Wo, dtype=np.float32))

    nc = _get_program()
    in_maps = []
    for c in range(NCORES):
        shards = []
        for g in range(NB * c, NB * c + NB):
            b, h = divmod(g, H)
            shards.append(x[b, RB * h:RB * h + RB, :])
        in_maps.append({
            "xs": np.ascontiguousarray(np.stack(shards, axis=0)),
            "wqkv": Wqkv,
            "wo": Wo,
        })

    res = run_bass_kernel_spmd(nc, in_maps, core_ids=list(range(NCORES)))
    y = np.empty((B, L, D), dtype=np.float32)
    for c in range(NCORES):
        ysc = res.results[c]["ys"]
        for idx, g in enumerate(range(NB * c, NB * c + NB)):
            b, h = divmod(g, H)
            y[b, RB * h:RB * h + RB, :] = ysc[idx]
    return y
